# revision 3
# baseline (speedup 1.0000x reference)
"""Multi-head causal attention (B=2, S=2048, HID=2048, H=16, D=128) on 8 TRN2
NeuronCores.

Sharding: core c handles batch b=c//4 and heads [4*(c%4) .. 4*(c%4)+3].
Each core computes qkv-projection + RoPE + causal attention + its partial
out-projection; the host sums the 4 partial outputs per batch (tensor-parallel
reduce) and stacks the 2 batches.

On-chip layout: all activations are kept transposed ([feature, token]) so the
whole chain runs on the PE array with no on-device transposes:
  qT/kT = W_qk^T-slice @ x^T   (RoPE applied during PSUM evacuation)
  S^T[k,q] = kT^T@qT ; A = exp(S^T*scale) (*causal mask)
  outT[d,q] = V^T-chunks @ A   (accumulated over k chunks)
  y[tok,col] = outT^T-chunks @ W_o-rows  (accumulated over heads)
Row-sums for the softmax denominator come from a ones-vector matmul in the
[k,q] layout; normalization uses a K=1 broadcast matmul.
Matmuls run in float32r (TF32-like, full PE rate at free-dim>=256).
"""
import sys

sys.path.insert(0, '/opt/trn_rl_repo')

import numpy as np

B, S, HID = 2, 2048, 2048
H, D = 16, 128
NH = H // 4          # heads per core = 4
HC = HID // 128      # hid chunks = 16
TB = 512             # token block for projection
NTB = S // TB        # 4
QB = 512             # q block in attention
NQB = S // QB        # 4
NKCH = S // 128      # k chunks total = 16
SCALE = 1.0 / float(np.sqrt(D))
BASE = 10000.0
N_CORES = 8

_cache = {}


def _build():
    import concourse.bass as bass  # noqa: F401
    import concourse.tile as tile
    from concourse import bacc, mybir

    f32 = mybir.dt.float32
    f32r = mybir.dt.float32r
    EXP = mybir.ActivationFunctionType.Exp
    MULT = mybir.AluOpType.mult
    ADD = mybir.AluOpType.add

    nc = bacc.Bacc("TRN2", target_bir_lowering=False, debug=False,
                   num_devices=N_CORES)

    xT = nc.dram_tensor("xT", [HID, S], f32r, kind="ExternalInput").ap()
    wqk = nc.dram_tensor("wqk", [HID, 2 * NH * D], f32r, kind="ExternalInput").ap()
    wv = nc.dram_tensor("wv", [HID, NH * D], f32r, kind="ExternalInput").ap()
    wo = nc.dram_tensor("wo", [NH * D, HID], f32r, kind="ExternalInput").ap()
    cosT = nc.dram_tensor("cosT", [D, S], f32, kind="ExternalInput").ap()
    sinS = nc.dram_tensor("sinS", [D, S], f32, kind="ExternalInput").ap()
    maskT = nc.dram_tensor("maskT", [128, 4 * QB], f32, kind="ExternalInput").ap()
    ones_col = nc.dram_tensor("ones_col", [128, 1], f32r, kind="ExternalInput").ap()
    ones_row = nc.dram_tensor("ones_row", [1, 128], f32r, kind="ExternalInput").ap()
    y = nc.dram_tensor("y", [S, HID], f32, kind="ExternalOutput").ap()

    with tile.TileContext(nc) as tc:
        with tc.tile_pool(name="persist", bufs=1) as pp:
            # persistent tiles
            qkT = [pp.tile([128, S], f32r, tag=f"qkT{i}", name=f"qkT{i}") for i in range(8)]
            v_all = pp.tile([128, NKCH * NH * D], f32r, tag="v_all")
            outT = [pp.tile([128, S], f32r, tag=f"outT{h}", name=f"outT{h}") for h in range(NH)]

            # ---- phase 1a: V projection (v_all[k-chunk, head, d]) ----
            with tc.tile_pool(name="p1a", bufs=1) as p1, \
                 tc.tile_pool(name="p1aw", bufs=1) as p1w, \
                 tc.tile_pool(name="ps1a", bufs=2, space="PSUM") as ps1:
                wvt = p1w.tile([128, HC * NH * D], f32r, tag="wvt")
                nc.sync.dma_start(
                    wvt[:].rearrange("p (c n) -> p c n", n=NH * D),
                    wv.rearrange("(c p) n -> p c n", p=128))
                for jb in range(NTB):
                    xTb = p1.tile([128, HC * TB], f32r, tag="xTb")
                    for c in range(HC):
                        nc.sync.dma_start(
                            xTb[:, c * TB:(c + 1) * TB],
                            xT[c * 128:(c + 1) * 128, jb * TB:(jb + 1) * TB])
                    for t2 in range(TB // 128):
                        cg = jb * (TB // 128) + t2  # global 128-token chunk
                        Pv = ps1.tile([128, NH * D], f32, tag="Pv")
                        for c in range(HC):
                            nc.tensor.matmul(
                                Pv[:],
                                xTb[:, c * TB + t2 * 128: c * TB + (t2 + 1) * 128],
                                wvt[:, c * NH * D:(c + 1) * NH * D],
                                start=(c == 0), stop=(c == HC - 1))
                        nc.scalar.copy(
                            v_all[:, cg * NH * D:(cg + 1) * NH * D], Pv[:])

            # ---- phase 1b: Q/K projection + RoPE ----
            with tc.tile_pool(name="p1b", bufs=1) as p1b, \
                 tc.tile_pool(name="p1bw", bufs=2) as p1bw, \
                 tc.tile_pool(name="rope", bufs=2) as rp, \
                 tc.tile_pool(name="trig", bufs=2) as trig, \
                 tc.tile_pool(name="ps1b", bufs=2, space="PSUM") as ps1b:
                for jb in range(NTB):
                    tcos = trig.tile([D, TB], f32, tag="tcos")
                    tsin = trig.tile([D, TB], f32, tag="tsin")
                    nc.sync.dma_start(tcos[:], cosT[:, jb * TB:(jb + 1) * TB])
                    nc.sync.dma_start(tsin[:], sinS[:, jb * TB:(jb + 1) * TB])
                    xTb = p1b.tile([128, HC * TB], f32r, tag="xTb2")
                    for c in range(HC):
                        nc.sync.dma_start(
                            xTb[:, c * TB:(c + 1) * TB],
                            xT[c * 128:(c + 1) * 128, jb * TB:(jb + 1) * TB])
                    for cc in range(8):  # 4 q cols then 4 k cols
                        wt = p1bw.tile([128, HC * 128], f32r, tag="wt")
                        nc.sync.dma_start(
                            wt[:].rearrange("p (c n) -> p c n", n=128),
                            wqk.rearrange("(c p) n -> p c n", p=128)[
                                :, :, cc * 128:(cc + 1) * 128])
                        P = ps1b.tile([128, TB], f32, tag="P")
                        for c in range(HC):
                            nc.tensor.matmul(
                                P[:], wt[:, c * 128:(c + 1) * 128],
                                xTb[:, c * TB:(c + 1) * TB],
                                start=(c == 0), stop=(c == HC - 1))
                        sl = slice(jb * TB, (jb + 1) * TB)
                        u = rp.tile([128, TB], f32, tag="u")
                        nc.scalar.copy(u[:], P[:])
                        rot = rp.tile([128, TB], f32, tag="rot")
                        nc.sync.dma_start(rot[0:64, :], u[64:128, :])
                        nc.sync.dma_start(rot[64:128, :], u[0:64, :])
                        m = rp.tile([128, TB], f32, tag="m")
                        nc.vector.tensor_tensor(
                            out=m[:], in0=rot[:], in1=tsin[:], op=MULT)
                        t = rp.tile([128, TB], f32, tag="t")
                        nc.vector.tensor_tensor(
                            out=t[:], in0=u[:], in1=tcos[:], op=MULT)
                        nc.vector.tensor_tensor(
                            out=qkT[cc][:, sl], in0=t[:], in1=m[:], op=ADD)

            # ---- phase 2: attention ----
            with tc.tile_pool(name="p23w", bufs=1) as p2w:
              wot = [p2w.tile([128, HID], f32r, tag=f"wot{h}", name=f"wot{h}") for h in range(NH)]
              for h in range(NH):
                  nc.sync.dma_start(wot[h][:], wo[h * 128:(h + 1) * 128, :])
              with tc.tile_pool(name="p2", bufs=4) as p2, \
                 tc.tile_pool(name="p2c", bufs=1) as p2c, \
                 tc.tile_pool(name="p2s", bufs=2) as p2s, \
                 tc.tile_pool(name="psS", bufs=3, space="PSUM") as psS, \
                 tc.tile_pool(name="psO", bufs=2, space="PSUM") as psO, \
                 tc.tile_pool(name="psR", bufs=2, space="PSUM") as psR, \
                 tc.tile_pool(name="psB", bufs=1, space="PSUM") as psB:
                tmask = p2c.tile([128, 4 * QB], f32, tag="tmask")
                nc.sync.dma_start(tmask[:], maskT[:])
                t1c = p2c.tile([128, 1], f32r, tag="t1c")
                t1r = p2c.tile([1, 128], f32r, tag="t1r")
                nc.sync.dma_start(t1c[:], ones_col[:])
                nc.sync.dma_start(t1r[:], ones_row[:])

                for h in range(NH):
                    qT_h, kT_h = qkT[h], qkT[NH + h]
                    for jb4 in range(NQB):
                        qsl = slice(jb4 * QB, (jb4 + 1) * QB)
                        O = psO.tile([128, QB], f32, tag="O")
                        R = psR.tile([1, QB], f32, tag="R")
                        nkc = (QB // 128) * (jb4 + 1)
                        for kc in range(nkc):
                            Sc = psS.tile([128, QB], f32, tag="S")
                            nc.tensor.matmul(
                                Sc[:], kT_h[:, kc * 128:(kc + 1) * 128],
                                qT_h[:, qsl], start=True, stop=True)
                            A = p2.tile([128, QB], f32r, tag="A")
                            mdiag = kc - (QB // 128) * jb4
                            if mdiag >= 0:  # diagonal 512x512 region
                                Araw = p2.tile([128, QB], f32, tag="Araw")
                                nc.scalar.activation(Araw[:], Sc[:], EXP,
                                                     scale=SCALE)
                                nc.vector.tensor_tensor(
                                    out=A[:], in0=Araw[:],
                                    in1=tmask[:, mdiag * QB:(mdiag + 1) * QB],
                                    op=MULT)
                            else:
                                nc.scalar.activation(A[:], Sc[:], EXP,
                                                     scale=SCALE)
                            nc.tensor.matmul(
                                O[:],
                                v_all[:, kc * NH * D + h * D:
                                      kc * NH * D + (h + 1) * D],
                                A[:], start=(kc == 0), stop=(kc == nkc - 1))
                            nc.tensor.matmul(
                                R[:], t1c[:], A[:],
                                start=(kc == 0), stop=(kc == nkc - 1))
                        rec = p2s.tile([1, QB], f32r, tag="rec")
                        with nc.allow_low_precision(reason="f32r recip"):
                            nc.vector.reciprocal(rec[:], R[:])
                        Bb = psB.tile([128, QB], f32, tag="B")
                        nc.tensor.matmul(Bb[:], t1r[:], rec[:],
                                         start=True, stop=True)
                        Bs = p2s.tile([128, QB], f32, tag="Bs")
                        nc.scalar.copy(Bs[:], Bb[:])
                        nc.vector.tensor_tensor(
                            out=outT[h][:, qsl], in0=O[:], in1=Bs[:], op=MULT)

              # ---- phase 3: out projection (partial) ----
              with tc.tile_pool(name="p3", bufs=3) as p3, \
                   tc.tile_pool(name="ps3", bufs=2, space="PSUM") as ps3:
                  for tch in range(S // 128):
                      for cb in range(HID // 512):
                          P3 = ps3.tile([128, 512], f32, tag="P3")
                          for h in range(NH):
                              nc.tensor.matmul(
                                  P3[:],
                                  outT[h][:, tch * 128:(tch + 1) * 128],
                                  wot[h][:, cb * 512:(cb + 1) * 512],
                                  start=(h == 0), stop=(h == NH - 1))
                          ys = p3.tile([128, 512], f32, tag="ys")
                          nc.vector.tensor_copy(ys[:], P3[:])
                          nc.sync.dma_start(
                              y[tch * 128:(tch + 1) * 128,
                                cb * 512:(cb + 1) * 512], ys[:])

    nc.compile()
    return nc


def _host_inputs(x, w_qkv, w_out):
    """Build the 8 per-core input maps."""
    # RoPE tables, transposed ([d, t]) with the rotate-half sign folded in.
    inv_freq = 1.0 / (BASE ** (np.arange(0, D, 2, dtype=np.float64) / D))
    pos = np.arange(S, dtype=np.float64)
    freqs = np.outer(inv_freq, pos)           # [64, S]
    cos_h = np.cos(freqs).astype(np.float32)
    sin_h = np.sin(freqs).astype(np.float32)
    cosT = np.concatenate([cos_h, cos_h], 0)  # [128, S]
    sinS = np.concatenate([-sin_h, sin_h], 0)  # signed sin

    # Causal masks for the 4 diagonal sub-blocks ([k-part, q-free])
    kp = np.arange(128)[:, None]
    qf = np.arange(QB)[None, :]
    maskT = np.concatenate(
        [(qf >= 128 * mm + kp).astype(np.float32) for mm in range(4)], axis=1)

    w3 = np.asarray(w_qkv, np.float32).reshape(HID, 3, H, D)
    wo_full = np.asarray(w_out, np.float32).reshape(H, D, HID)
    x = np.asarray(x, np.float32)

    shared = {
        "cosT": cosT, "sinS": sinS, "maskT": maskT,
        "ones_col": np.ones((128, 1), np.float32),
        "ones_row": np.ones((1, 128), np.float32),
    }
    in_maps = []
    for c in range(N_CORES):
        b, hg = c // 4, c % 4
        heads = slice(4 * hg, 4 * hg + 4)
        wqk = np.ascontiguousarray(
            w3[:, 0:2, heads, :].reshape(HID, 2 * NH * D))
        wv = np.ascontiguousarray(w3[:, 2, heads, :].reshape(HID, NH * D))
        wo_c = np.ascontiguousarray(wo_full[heads].reshape(NH * D, HID))
        in_maps.append({
            "xT": np.ascontiguousarray(x[b].T),
            "wqk": wqk, "wv": wv, "wo": wo_c, **shared,
        })
    return in_maps


def kernel(x, w_qkv, w_out):
    from concourse.bass_utils import run_bass_kernel_spmd

    if "nc" not in _cache:
        _cache["nc"] = _build()
    nc = _cache["nc"]
    in_maps = _host_inputs(x, w_qkv, w_out)
    res = run_bass_kernel_spmd(nc, in_maps, core_ids=list(range(N_CORES)))
    out = np.zeros((B, S, HID), np.float32)
    for c in range(N_CORES):
        out[c // 4] += res.results[c]["y"]
    return out
